# revision 4
# baseline (speedup 1.0000x reference)
"""Distributed CLIP-style batch contrastive loss on 8 Trainium2 NeuronCores.

Single-slab fp8 redesign (v2).  Math (B=8192, D=256, temp=0.07):
    sim = tn @ gn.T / temp          [B, B]
    t2g = mean(LSE_row) - mean(diag)/temp ; g2t = mean(LSE_col) - same
    accs = mean(diag is row/col argmax)

Each core owns 1024 rows of both modalities.  It normalizes both shards
(scaled by 4 so fp8-e4m3 keeps ~N(0,1) operands), transposes them
d-major via the PE, AllGathers g in fp8 (the only large collective),
then computes ONE [1024, 8192] similarity slab with DoubleRow fp8
matmuls (K=256 per instruction, one matmul per 512-col chunk).
Row (t2g) statistics are local: ACT evaluates exp(sim/temp/16) over
2048-wide PSUM spans with accum_out row sums.  Column (g2t) statistics
ride sums instead of maxes so they can use the PE + AllReduce(add):
  * col sums: bf16 running colacc (DVE tensor_tensor, 2x mode) for
    m=0..6 plus the last e tile, partition-summed by replicated
    ones-matvecs on the PE.
  * col argmax: the exceedance sum  sum_i relu(e_ij - thr_j)  is zero
    iff no element beats the (fudged) diagonal of column j; thr_j comes
    from a tiny diag AllGather broadcast into thrfull.
  * row argmax: max-fold e to 1024 wide (first fold on GpSimd), reduce,
    relu vs thr_i, then min(x*1e30, 1) as an exact 0/1 indicator.
One AllReduce at the end carries [colsum | colrelu | scalars]; every
core finishes the scalar math redundantly (pure SPMD, no core id).
"""

import numpy as np

import concourse.bacc as bacc
import concourse.bass as bass
import concourse.mybir as mybir
import concourse.tile as tile
from concourse import masks
from concourse.bass_utils import run_bass_kernel_spmd

B = 8192
D = 256
NCORES = 8
BS = B // NCORES          # 1024 rows per core
MT = BS // 128            # 8 m-tiles per shard
SPAN = 2048               # exp span = 4 psum banks
NSP = B // SPAN           # 4 spans per m-tile row
NWARM = 10                # spaced dummy matmuls keep the PE HAM alive
TEMP = 0.07
INV_TEMP = 1.0 / TEMP
SCALE = 1.0
PSCL = SCALE * SCALE
FUDGE = 1.004             # rounding margin for the argmax compares
BIG = 1.0e30              # exact 0/1 indicator: min(relu*BIG, 1)
GCOLS = 1024              # colmax stripe owned by GpSimd

F32 = mybir.dt.float32
BF16 = mybir.dt.bfloat16
AX = mybir.AxisListType.X
OP = mybir.AluOpType
AF = mybir.ActivationFunctionType


def emit(ctx, tc, t_in, g_in, out5):
    nc = tc.nc
    rg = [list(range(NCORES))]

    consts = ctx.enter_context(tc.tile_pool(name="consts", bufs=1))
    ident_bf = consts.tile([128, 128], BF16, name="ident_bf")
    masks.make_identity(nc, ident_bf)
    ones128 = consts.tile([128, 128], BF16, name="ones128")
    nc.vector.memset(ones128, 1.0)
    ones_f32 = consts.tile([128, 1], F32, name="ones_f32")
    nc.vector.memset(ones_f32, 1.0)
    pw_rhs = consts.tile([128, 512], BF16, name="pw_rhs")
    nc.vector.memset(pw_rhs, 0.0)

    # persistent SBUF tensors
    pers = ctx.enter_context(tc.tile_pool(name="pers", bufs=1))
    tdm = [pers.tile([128, BS], BF16, name=f"tdm{h}") for h in range(2)]
    gdm = [pers.tile([128, BS], BF16, name=f"gdm{h}") for h in range(2)]
    gfull = [pers.tile([128, B], BF16, name=f"gfull{h}") for h in range(2)]
    gball = pers.tile([128, MT * D], BF16, name="gball")
    ebuf = [pers.tile([128, B], BF16, name=f"ebuf{h}") for h in range(3)]
    colacc = pers.tile([128, B], BF16, name="colacc")
    colmax = pers.tile([128, B], BF16, name="colmax")
    thrfull = pers.tile([128, B], BF16, name="thrfull")
    crel = pers.tile([128, B], BF16, name="crel")
    dbuf = pers.tile([128, MT], F32, name="dbuf")       # 16 * diag cos
    thr = pers.tile([128, MT], F32, name="thr")         # exp(diag/T)*fudge
    negthr = pers.tile([128, MT], F32, name="negthr")
    ediagf = pers.tile([128, MT], BF16, name="ediagf")  # same, bf16 for AG
    rsg = pers.tile([128, MT, NSP], F32, name="rsg")    # row sum grid
    rmaxg = pers.tile([128, MT], F32, name="rmaxg")     # row max grid
    drain = pers.tile([128, SPAN], F32, name="drain")   # psum drain scratch
    thrflat = pers.tile([1, B], BF16, name="thrflat")
    sca = pers.tile([128, 4], F32, name="sca")          # packed scalars

    # DRAM tiles for collectives + bounce
    dram = ctx.enter_context(tc.tile_pool(name="dram", bufs=1, space="DRAM"))
    dumin = dram.tile([1, 16], BF16, name="dumin")
    dumout = dram.tile([NCORES, 16], BF16, name="dumout", addr_space="Shared")
    agin_g = dram.tile([2 * 128, BS], BF16, name="agin_g")
    agout_g = dram.tile([NCORES * 2 * 128, BS], BF16, name="agout_g",
                        addr_space="Shared")
    agin_d = dram.tile([MT, 128], BF16, name="agin_d")
    agout_d = dram.tile([NCORES * MT, 128], BF16, name="agout_d",
                        addr_space="Shared")
    thrbounce = dram.tile([1, B], BF16, name="thrbounce")
    arin = dram.tile([1, 2 * B + 8], F32, name="arin")
    arout = dram.tile([1, 2 * B + 8], F32, name="arout", addr_space="Shared")

    # dummy collective: absorbs the cross-core launch-skew entry barrier
    dum_s = consts.tile([1, 16], BF16, name="dum_s")
    nc.vector.memset(dum_s, 0.0)
    nc.sync.dma_start(dumin, dum_s)
    nc.gpsimd.collective_compute("AllGather", OP.bypass, replica_groups=rg,
                                 ins=[dumin.opt()], outs=[dumout.opt()])
    dum_r = consts.tile([1, 16], BF16, name="dum_r")
    nc.sync.dma_start(dum_r, dumout[0:1, :])

    # ---------- phase 1: normalize shards, diag, transpose, gathers --------
    with tc.tile_pool(name="nat", bufs=4) as natp, \
         tc.tile_pool(name="sq", bufs=3) as sqp, \
         tc.tile_pool(name="small", bufs=8) as smallp, \
         tc.tile_pool(name="nrm", bufs=4) as nrmp, \
         tc.tile_pool(name="tpsum", bufs=4, space="PSUM") as tpsum, \
         tc.tile_pool(name="wpsum", bufs=1, space="PSUM") as wpsum:

        def normalize(src_slice, dst_bf):
            nat = natp.tile([128, D], F32, name="nat", tag="nat")
            nc.sync.dma_start(nat, src_slice)
            sq = sqp.tile([128, D], F32, name="sq", tag="sq")
            ss = smallp.tile([128, 1], F32, name="ss", tag="ss")
            nc.scalar.activation(sq, nat, AF.Square, accum_out=ss)
            nrmv = smallp.tile([128, 1], F32, name="nrmv", tag="nrmv")
            nc.scalar.activation(nrmv, ss, AF.Sqrt)
            rn = smallp.tile([128, 1], F32, name="rn", tag="rn")
            nc.vector.reciprocal(rn, nrmv)
            nc.vector.tensor_scalar_mul(dst_bf, nat, rn)

        def transpose_to(nrm_bf, dst_of_h):
            for h in range(2):
                pt = tpsum.tile([128, 128], BF16, name="pt", tag="pt")
                nc.tensor.transpose(pt, nrm_bf[:, h * 128:(h + 1) * 128],
                                    ident_bf)
                nc.vector.tensor_copy(dst_of_h(h), pt)

        # g first so its AllGather flies while t is being prepped
        for m in range(MT):
            gb = gball[:, m * D:(m + 1) * D]
            normalize(g_in[m * 128:(m + 1) * 128, :], gb)
            transpose_to(
                gb,
                lambda h, m=m: gdm[h][:, m * 128:(m + 1) * 128])

        for h in range(2):
            nc.sync.dma_start(agin_g[h * 128:(h + 1) * 128, :], gdm[h])
        nc.gpsimd.collective_compute(
            "AllGather", OP.bypass, replica_groups=rg,
            ins=[agin_g.opt()], outs=[agout_g.opt()])
        for r in range(NCORES):
            for h in range(2):
                nc.sync.dma_start(
                    gfull[h][:, r * BS:(r + 1) * BS],
                    agout_g[r * 256 + h * 128:r * 256 + (h + 1) * 128, :])

        for m in range(MT):
            tb = nrmp.tile([128, D], BF16, name="tb", tag="tb")
            normalize(t_in[m * 128:(m + 1) * 128, :], tb)
            # local diagonal block: 16 * (tn_i . gn_i)
            dsc = sqp.tile([128, D], F32, name="dsc", tag="sq")
            nc.vector.tensor_tensor(dsc, tb, gball[:, m * D:(m + 1) * D],
                                    OP.mult)
            nc.vector.reduce_sum(out=dbuf[:, m:m + 1], in_=dsc, axis=AX)
            transpose_to(
                tb, lambda h, m=m: tdm[h][:, m * 128:(m + 1) * 128])

        # thresholds: thr = exp(diag/T)*FUDGE (f32 rows + bf16 for the AG)
        nc.scalar.activation(thr, dbuf, AF.Exp, scale=INV_TEMP / PSCL)
        nc.vector.tensor_scalar_mul(thr, thr, FUDGE)
        nc.vector.tensor_scalar(negthr, thr, -1.0, None, OP.mult)
        nc.vector.tensor_copy(ediagf, thr)
        nc.sync.dma_start(agin_d.rearrange("m p -> p m"), ediagf)
        nc.gpsimd.collective_compute(
            "AllGather", OP.bypass, replica_groups=rg,
            ins=[agin_d.opt()], outs=[agout_d.opt()])
        # agout_d is flat j-order (c, m, p); bounce and broadcast
        nc.sync.dma_start(thrflat, agout_d)
        nc.sync.dma_start(thrbounce, thrflat)
        nc.sync.dma_start(thrfull, thrbounce.broadcast_to([128, B]))

        # keep the PE HAM alive with a low-duty trickle (a dense burst
        # trips the SW utilization throttler and poisons the slab)
        pw_sb = consts.tile([128, 512], BF16, name="pw_sb")
        for _ in range(NWARM):
            pw_ps = wpsum.tile([128, 512], F32, name="pw_ps", tag="pw")
            nc.tensor.matmul(pw_ps, lhsT=ident_bf, rhs=pw_rhs,
                             start=True, stop=True)
            nc.vector.tensor_copy(pw_sb, pw_ps)

    # ---------- phase 2: the slab ----------
    CM0 = B - GCOLS
    with tc.tile_pool(name="spsum", bufs=2, space="PSUM") as spsum:
        for m in range(MT):
            e = ebuf[m % 3]
            for sp in range(NSP):
                ps = spsum.tile([128, SPAN], F32, name="ps", tag="ps")
                for k in range(2):
                    for c4 in range(SPAN // 512):
                        n0 = sp * SPAN + c4 * 512
                        nc.tensor.matmul(
                            ps[:, c4 * 512:(c4 + 1) * 512],
                            lhsT=tdm[k][:, m * 128:(m + 1) * 128],
                            rhs=gfull[k][:, n0:n0 + 512],
                            start=(k == 0), stop=(k == 1))
                nc.scalar.activation(e[:, sp * SPAN:(sp + 1) * SPAN], ps,
                                     AF.Exp, scale=INV_TEMP / PSCL,
                                     accum_out=rsg[:, m, sp:sp + 1])
            # column stats (bf16 2x tensor_tensor; GpSimd owns the last
            # GCOLS of colmax).  m=7's colacc is folded into the PE
            # matvec below instead.
            if m == 0:
                nc.vector.tensor_copy(colacc, e)
                nc.vector.tensor_copy(colmax, e)
            else:
                if m < MT - 1:
                    nc.vector.tensor_tensor(colacc, colacc, e, OP.add)
                nc.vector.tensor_tensor(colmax, colmax, e, OP.max)
            # row exceedance on ACT's slack: accum = sum_j relu(e - thr_i)
            nc.scalar.activation(crel, e, AF.Relu, bias=negthr[:, m:m + 1],
                                 accum_out=rmaxg[:, m:m + 1])
    # ---------- phase 3: finish + single AllReduce ----------
    e7 = ebuf[(MT - 1) % 3]
    with tc.tile_pool(name="fin", bufs=2) as finp, \
         tc.tile_pool(name="gpsum", bufs=2, space="PSUM") as gpsum:
        # colsum matvecs (PE) run in parallel with crel (DVE)
        for r in range(4):
            fp = gpsum.tile([128, SPAN], F32, name="gp", tag="gp")
            for c4 in range(4):
                n0 = r * SPAN + c4 * 512
                sl = fp[:, c4 * 512:(c4 + 1) * 512]
                nc.tensor.matmul(sl, lhsT=ones128,
                                 rhs=colacc[:, n0:n0 + 512],
                                 start=True, stop=False)
                nc.tensor.matmul(sl, lhsT=ones128, rhs=e7[:, n0:n0 + 512],
                                 start=False, stop=True)
            nc.vector.tensor_copy(drain, fp)
            nc.sync.dma_start(arin[0:1, r * SPAN:(r + 1) * SPAN],
                              drain[0:1, :])
        # column exceedance tile: relu(colmax - thrfull)
        nc.vector.tensor_tensor(crel, colmax, thrfull, OP.subtract)
        nc.vector.tensor_scalar(crel, crel, 0.0, None, OP.max)
        for r in range(4):
            fp = gpsum.tile([128, SPAN], F32, name="gp", tag="gp")
            for c4 in range(4):
                n0 = r * SPAN + c4 * 512
                nc.tensor.matmul(fp[:, c4 * 512:(c4 + 1) * 512],
                                 lhsT=ones128, rhs=crel[:, n0:n0 + 512],
                                 start=True, stop=True)
            nc.vector.tensor_copy(drain, fp)
            nc.sync.dma_start(arin[0:1, B + r * SPAN:B + (r + 1) * SPAN],
                              drain[0:1, :])

        # row scalars: sum_i ln(rowsum_i), sum diag, count rows exceeded
        rowsum = finp.tile([128, MT], F32, name="rowsum")
        nc.vector.reduce_sum(out=rowsum, in_=rsg, axis=AX)
        lnrs = finp.tile([128, MT], F32, name="lnrs")
        nc.scalar.activation(lnrs, rowsum, AF.Ln)
        nc.vector.reduce_sum(out=sca[:, 0:1], in_=lnrs, axis=AX)
        nc.vector.reduce_sum(out=sca[:, 1:2], in_=dbuf, axis=AX)
        rrel = finp.tile([128, MT], F32, name="rrel")
        nc.vector.tensor_scalar(rrel, rmaxg, BIG, 1.0, OP.mult, OP.min)
        nc.vector.reduce_sum(out=sca[:, 2:3], in_=rrel, axis=AX)
        # partition reduce the 3 scalars on the PE (f32 matvec, tiny)
        pss = gpsum.tile([1, 8], F32, name="pss", tag="gp")
        nc.tensor.matmul(pss[0:1, 0:3], lhsT=ones_f32, rhs=sca[:, 0:3],
                         start=True, stop=True)
        sc8 = finp.tile([1, 8], F32, name="sc8")
        nc.vector.memset(sc8, 0.0)
        nc.vector.tensor_copy(sc8[0:1, 0:3], pss[0:1, 0:3])
        nc.sync.dma_start(arin[0:1, 2 * B:2 * B + 8], sc8)

        nc.gpsimd.collective_compute(
            "AllReduce", OP.add, replica_groups=rg,
            ins=[arin.opt()], outs=[arout.opt()])

        # ---------- phase 4: post-AR finalize (identical on every core) ----
        csT = finp.tile([128, 64], F32, name="csT")
        nc.sync.dma_start(
            csT,
            arout[0:1, 0:B].rearrange("o (p j) -> o p j", p=128).squeeze(0))
        crT = finp.tile([128, 64], F32, name="crT")
        nc.sync.dma_start(
            crT,
            arout[0:1, B:2 * B].rearrange("o (p j) -> o p j",
                                          p=128).squeeze(0))
        scr = finp.tile([1, 8], F32, name="scr")
        nc.sync.dma_start(scr, arout[0:1, 2 * B:2 * B + 8])

        lncs = finp.tile([128, 64], F32, name="lncs")
        colsc = finp.tile([128, 2], F32, name="colsc")
        nc.scalar.activation(lncs, csT, AF.Ln, accum_out=colsc[:, 0:1])
        nc.vector.tensor_scalar(crT, crT, BIG, 1.0, OP.mult, OP.min)
        nc.vector.reduce_sum(out=colsc[:, 1:2], in_=crT, axis=AX)
        psc = gpsum.tile([1, 8], F32, name="psc", tag="gp")
        nc.tensor.matmul(psc[0:1, 0:2], lhsT=ones_f32, rhs=colsc,
                         start=True, stop=True)

        # final scalar math on one lane
        res = finp.tile([1, 5], F32, name="res")
        tmp = finp.tile([1, 4], F32, name="tmp")
        # tmp0 = mean diag / temp ; tmp1 = t2g lse mean ; tmp2 = g2t
        nc.vector.tensor_scalar(tmp[0:1, 0:1], scr[0:1, 1:2],
                                INV_TEMP / PSCL / B, None, OP.mult)
        nc.vector.tensor_scalar(tmp[0:1, 1:2], scr[0:1, 0:1],
                                1.0 / B, None, OP.mult)
        nc.vector.tensor_scalar(tmp[0:1, 2:3], psc[0:1, 0:1],
                                1.0 / B, None, OP.mult)
        nc.vector.tensor_tensor(res[0:1, 1:2], tmp[0:1, 1:2], tmp[0:1, 0:1],
                                OP.subtract)
        nc.vector.tensor_tensor(res[0:1, 2:3], tmp[0:1, 2:3], tmp[0:1, 0:1],
                                OP.subtract)
        nc.vector.tensor_tensor(res[0:1, 0:1], res[0:1, 1:2], res[0:1, 2:3],
                                OP.add)
        # accs = (B - exceed_count)/B = 1 - count/B
        nc.vector.tensor_scalar(res[0:1, 3:4], scr[0:1, 2:3],
                                -1.0 / B, 1.0, OP.mult, OP.add)
        nc.vector.tensor_scalar(res[0:1, 4:5], psc[0:1, 1:2],
                                -1.0 / B, 1.0, OP.mult, OP.add)
        nc.sync.dma_start(out5, res)


_CACHE = {}


def build():
    if "nc" in _CACHE:
        return _CACHE["nc"]
    import contextlib
    nc = bacc.Bacc("TRN2", target_bir_lowering=False, debug=False,
                   enable_asserts=False, num_devices=NCORES)
    t_in = nc.dram_tensor("t_shard", [BS, D], F32, kind="ExternalInput").ap()
    g_in = nc.dram_tensor("g_shard", [BS, D], F32, kind="ExternalInput").ap()
    out5 = nc.dram_tensor("out5", [1, 5], F32, kind="ExternalOutput").ap()
    with tile.TileContext(nc) as tc:
        with contextlib.ExitStack() as ctx:
            emit(ctx, tc, t_in, g_in, out5)
    nc.compile()
    _CACHE["nc"] = nc
    return nc


def kernel(text_embeddings, graph_embeddings, **_):
    t = np.ascontiguousarray(np.asarray(text_embeddings, dtype=np.float32))
    g = np.ascontiguousarray(np.asarray(graph_embeddings, dtype=np.float32))
    assert t.shape == (B, D) and g.shape == (B, D)
    nc = build()
    in_maps = [
        {"t_shard": t[k * BS:(k + 1) * BS], "g_shard": g[k * BS:(k + 1) * BS]}
        for k in range(NCORES)
    ]
    res = run_bass_kernel_spmd(nc, in_maps, core_ids=list(range(NCORES)))
    return res.results[0]["out5"].reshape(5).astype(np.float32)


if __name__ == "__main__":
    rng = np.random.default_rng(0)
    t = rng.standard_normal((B, D), dtype=np.float32)
    g = rng.standard_normal((B, D), dtype=np.float32)
    print(kernel(text_embeddings=t, graph_embeddings=g))
